# revision 1
# baseline (speedup 1.0000x reference)
"""Trainium2 Bass kernel for multi-head attention (nn_Attention).

Problem: x[8, 32, 32, 768] -> MHA(12 heads, d=64) -> out[8, 32, 32, 768].

Sharding: pure data parallel. Batch B=8 maps 1:1 onto the 8 NeuronCores;
weights are replicated. No collectives.

Per-core algorithm (N=1024 tokens, C=768), all matmuls bf16 with fp32 PSUM
accumulation. Emission interleaves the QKV projection with the attention
head pairs so the ScalarE exp stream starts ~20us in and overlaps all of
the PE work:
  1. DMA x/qkv_w row tiles, DVE-cast bf16, PE-transpose to feature-major
     xT[c,n] / WT[c,o] (contraction dim must live on SBUF partitions).
  2. qT/kT (feature-major) = WT.T @ xT per head pair, just before that
     pair's scores;  V (token-major) = xT.T @ WT_v between pairs 0 and 1.
  3. Per pair: S^T[j,i] = kT.T @ qT (K=64, two heads packed into the PE
     array via tile_position).  E = exp(S^T/8) via ACT from PSUM (no
     max-subtraction: scores ~ N(0,1)).
  4. PV (overlapped with the next pair's scores): out^T[d,i] + denominator
     row (ones-column of V) = [V|1].T @ E — no P-matrix transpose.
  5. Normalize by 1/denom: fast-approx reciprocal, fp32r PE ones-broadcast
     (bf16 k=1 weight loads corrupt on HW), DVE multiply into O^T.
  6. out = O^T.T @ PwT + proj_b, DMA out per token tile.
"""

import os
import sys

for _p in ("/opt/trn_rl_repo",):
    if _p not in sys.path:
        sys.path.insert(0, _p)

import numpy as np

import concourse.bass as bass
from concourse import bacc
import concourse.mybir as mybir
from concourse.masks import make_identity
from concourse.tile import TileContext

F32 = mybir.dt.float32
F32R = mybir.dt.float32r
BF16 = mybir.dt.bfloat16

P = 128
C = 768            # model dim
CT = C // P        # 6 c-tiles
N = 1024           # tokens per batch element
NT = N // P        # 8 token tiles
HEADS = 12
D = 64
OQK = 2 * C        # 1536 rows of q+k features
OTQK = OQK // P    # 12
OT3 = 3 * C // P   # 18 qkv_w row tiles
SCALE = D ** -0.5  # 0.125


def build_nc() -> bass.Bass:
    nc = bacc.Bacc(None, target_bir_lowering=False)
    x_d = nc.declare_dram_parameter("x", [N, C], F32, isOutput=False)
    qkvw_d = nc.declare_dram_parameter("qkv_w", [3 * C, C], F32, isOutput=False)
    qkvb_d = nc.declare_dram_parameter("qkv_b", [3 * C], F32, isOutput=False)
    projw_d = nc.declare_dram_parameter("proj_w", [C, C], F32, isOutput=False)
    projb_d = nc.declare_dram_parameter("proj_b", [C], F32, isOutput=False)
    out_d = nc.declare_dram_parameter("out", [N, C], F32, isOutput=True)

    with TileContext(nc) as tc:
        with (
            tc.tile_pool(name="const", bufs=1) as cpool,
            tc.tile_pool(name="load", bufs=2) as lpool,
            tc.tile_pool(name="ldb", bufs=2) as lbpool,
            tc.tile_pool(name="qk", bufs=1) as qkpool,
            tc.tile_pool(name="v", bufs=1) as vpool,
            tc.tile_pool(name="otp", bufs=1) as otpool,
            tc.tile_pool(name="xTp", bufs=1) as xtpool,
            tc.tile_pool(name="wTp", bufs=1) as wtpool,
            tc.tile_pool(name="pwp", bufs=1) as pwpool,
            tc.tile_pool(name="e", bufs=4) as epool,
            tc.tile_pool(name="rec", bufs=1) as rpool,
            tc.tile_pool(name="outs", bufs=2) as outpool,
            tc.tile_pool(name="psa", bufs=3, space="PSUM") as psa,
            tc.tile_pool(name="psb", bufs=1, space="PSUM") as psb,
        ):
            ident = cpool.tile([P, P], F32, tag="ident")
            make_identity(nc, ident)
            ones_st = cpool.tile([1, P], F32, tag="ones_st")
            nc.gpsimd.memset(ones_st, 1.0)
            ones_row = cpool.tile([1, P], BF16, tag="ones")
            nc.vector.tensor_copy(ones_row, ones_st)
            ones_r = cpool.tile([1, P], F32R, tag="ones_r")
            nc.vector.tensor_copy(ones_r, ones_st)

            # Biases. q/k bias is applied per-partition (feature-major);
            # v/proj biases seed the PSUM accumulation via a ones-outer-
            # product matmul (free-dim broadcast).
            bqk = cpool.tile([P, OTQK], F32, tag="bqk")
            nc.sync.dma_start(bqk, qkvb_d[0:OQK].rearrange("(t p) -> p t", p=P))
            bv_st = cpool.tile([1, C], F32, tag="bv_st")
            nc.sync.dma_start(bv_st, qkvb_d[None, OQK : 3 * C])
            bv = cpool.tile([1, C], BF16, tag="bv")
            nc.vector.tensor_copy(bv, bv_st)
            pb_st = cpool.tile([1, C], F32, tag="pb_st")
            nc.sync.dma_start(pb_st, projb_d[None, :])
            pb = cpool.tile([1, C], BF16, tag="pb")
            nc.vector.tensor_copy(pb, pb_st)

            # Persistent activations
            qkT = qkpool.tile([P, OTQK, N], BF16, tag="qkT")      # q,k feature-major
            V = vpool.tile([P, NT, HEADS, D + 1], BF16, tag="V")  # token-major + ones col
            OT = otpool.tile([P, CT, N], BF16, tag="OT")          # attn out, feature-major
            xT = xtpool.tile([P, CT, N], BF16, tag="xT")
            WT = wtpool.tile([P, CT, 3 * C], BF16, tag="WT")
            PwT = pwpool.tile([P, CT, C], BF16, tag="PwT")

            nc.gpsimd.memset(V[:, :, :, D], 1.0)

            def load_cast_transpose(dram_row_tile, dest, dest_block):
                """DMA a [128, C] fp32 row tile, fp32 PE-transpose the six
                [128,128] blocks, casting to bf16 on the PSUM->SBUF copy."""
                st = lpool.tile([P, C], F32, tag="ld")
                nc.sync.dma_start(st, dram_row_tile)
                for ct in range(CT):
                    ps = psa.tile([P, P], F32, tag="psa")
                    nc.tensor.transpose(ps, st[:, ct * P : (ct + 1) * P], ident)
                    nc.vector.tensor_copy(
                        dest[:, ct, dest_block * P : (dest_block + 1) * P], ps
                    )

            def qkv_w_tile(ot):
                load_cast_transpose(qkvw_d[ot * P : (ot + 1) * P, :], WT, ot)

            def qkv_qk(ot):
                """Feature-major q/k projection for one 128-feature tile."""
                ps = psa.tile([P, N], F32, tag="psa", name="ps_qk")
                for ic in range(2):
                    for ct in range(CT):
                        nc.tensor.matmul(
                            ps[:, ic * 512 : (ic + 1) * 512],
                            WT[:, ct, ot * P : (ot + 1) * P],
                            xT[:, ct, ic * 512 : (ic + 1) * 512],
                            start=(ct == 0),
                            stop=(ct == CT - 1),
                        )
                nc.vector.tensor_scalar_add(qkT[:, ot, :], ps, bqk[:, ot : ot + 1])

            def v_proj(nt):
                """Token-major V projection (bias-seeded) for one token tile."""
                ps = psb.tile([P, N], F32, tag="psb", name="ps_v")
                for o0, ow in ((0, 512), (512, 256)):
                    pss = ps[:, o0 : o0 + ow]
                    nc.tensor.matmul(
                        pss, ones_row, bv[:, o0 : o0 + ow], start=True, stop=False
                    )
                    for ct in range(CT):
                        nc.tensor.matmul(
                            pss,
                            xT[:, ct, nt * P : (nt + 1) * P],
                            WT[:, ct, OQK + o0 : OQK + o0 + ow],
                            start=False,
                            stop=(ct == CT - 1),
                        )
                nc.vector.tensor_copy(
                    V[:, nt, :, 0:D], ps[:, :C].rearrange("p (h d) -> p h d", d=D)
                )

            def scores_pair(pair):
                E0 = epool.tile([P, NT, N], BF16, tag="E", name="E0")
                E1 = epool.tile([P, NT, N], BF16, tag="E", name="E1")
                for jt in range(NT):
                    for half, E in ((0, E0), (1, E1)):
                        lo, hi = half * D, half * D + D
                        ps = psa.tile([P, N], F32, tag="psa", name="ps_s")
                        for ic in range(2):
                            nc.tensor.matmul(
                                ps[:, ic * 512 : (ic + 1) * 512],
                                qkT[lo:hi, OTQK // 2 + pair, jt * P : (jt + 1) * P],
                                qkT[lo:hi, pair, ic * 512 : (ic + 1) * 512],
                                start=True,
                                stop=True,
                                tile_position=(half * D, 0),
                            )
                        nc.scalar.activation(
                            E[:, jt, :], ps, mybir.ActivationFunctionType.Exp, scale=SCALE
                        )
                return E0, E1

            def pv_head(h, E):
                pspv = psb.tile([P, N], F32, tag="psb", name="ps_pv")
                for ic in range(2):
                    for jt in range(NT):
                        nc.tensor.matmul(
                            pspv[0 : D + 1, ic * 512 : (ic + 1) * 512],
                            V[:, jt, h, :],
                            E[:, jt, ic * 512 : (ic + 1) * 512],
                            start=(jt == 0),
                            stop=(jt == NT - 1),
                        )
                den_sb = rpool.tile([1, N], F32, tag="den_sb")
                nc.vector.tensor_copy(den_sb, pspv[D : D + 1, :])
                rec_st = rpool.tile([1, N], F32, tag="rec_st")
                nc.vector.reciprocal_approx_fast(rec_st, den_sb)
                rec = rpool.tile([1, N], F32R, tag="rec")
                nc.vector.tensor_copy(rec, rec_st)
                psbc = psa.tile([P, N], F32, tag="psa", name="ps_bc")
                for ic in range(2):
                    nc.tensor.matmul(
                        psbc[:, ic * 512 : (ic + 1) * 512],
                        ones_r,
                        rec[:, ic * 512 : (ic + 1) * 512],
                        start=True,
                        stop=True,
                    )
                bcast = rpool.tile([D, N], BF16, tag="bc")
                nc.vector.tensor_copy(bcast, psbc[0:D, :])
                nc.vector.tensor_mul(
                    OT[(h % 2) * D : (h % 2) * D + D, h // 2, :], pspv[0:D, :], bcast
                )

            def proj_tile(it):
                outt = outpool.tile([P, C], F32, tag="out")
                ps = psa.tile([P, N], F32, tag="psa", name="ps_o")
                for o0, ow in ((0, 512), (512, 256)):
                    pss = ps[:, o0 : o0 + ow]
                    nc.tensor.matmul(
                        pss, ones_row, pb[:, o0 : o0 + ow], start=True, stop=False
                    )
                    for ct in range(CT):
                        nc.tensor.matmul(
                            pss,
                            OT[:, ct, it * P : (it + 1) * P],
                            PwT[:, ct, o0 : o0 + ow],
                            start=False,
                            stop=(ct == CT - 1),
                        )
                nc.vector.tensor_copy(outt, ps[:, :C])
                nc.sync.dma_start(out_d[it * P : (it + 1) * P, :], outt)

            # ---------------- interleaved emission ----------------
            for nt in range(NT):
                load_cast_transpose(x_d[nt * P : (nt + 1) * P, :], xT, nt)

            Es = {}
            qkv_w_tile(0)
            qkv_w_tile(6)
            qkv_qk(0)
            qkv_qk(6)
            Es[0] = scores_pair(0)

            qkv_w_tile(1)
            qkv_w_tile(7)
            qkv_qk(1)
            qkv_qk(7)
            for ot in (12, 13, 14, 15, 16, 17):
                qkv_w_tile(ot)
            for nt in (0, 1, 2, 3):
                v_proj(nt)
            Es[1] = scores_pair(1)

            for nt in (4, 5, 6, 7):
                v_proj(nt)
            pv_head(0, Es[0][0])
            pv_head(1, Es[0][1])

            qkv_w_tile(2)
            qkv_w_tile(8)
            qkv_qk(2)
            qkv_qk(8)
            Es[2] = scores_pair(2)
            pv_head(2, Es[1][0])
            pv_head(3, Es[1][1])

            qkv_w_tile(3)
            qkv_w_tile(9)
            qkv_qk(3)
            qkv_qk(9)
            Es[3] = scores_pair(3)
            pv_head(4, Es[2][0])
            pv_head(5, Es[2][1])

            for ot in range(CT):
                load_cast_transpose(projw_d[ot * P : (ot + 1) * P, :], PwT, ot)

            qkv_w_tile(4)
            qkv_w_tile(10)
            qkv_qk(4)
            qkv_qk(10)
            Es[4] = scores_pair(4)
            pv_head(6, Es[3][0])
            pv_head(7, Es[3][1])

            qkv_w_tile(5)
            qkv_w_tile(11)
            qkv_qk(5)
            qkv_qk(11)
            Es[5] = scores_pair(5)
            pv_head(8, Es[4][0])
            pv_head(9, Es[4][1])

            pv_head(10, Es[5][0])
            pv_head(11, Es[5][1])

            for it in range(NT):
                proj_tile(it)

    nc.compile()
    return nc


_NC_CACHE = None


def _get_nc():
    global _NC_CACHE
    if _NC_CACHE is None:
        _NC_CACHE = build_nc()
    return _NC_CACHE


def run(inputs, trace=False, tmpdir=None):
    """Run on 8 NeuronCores; returns (out[8,32,32,768], BassKernelResults)."""
    from concourse.bass_utils import run_bass_kernel_spmd

    x = np.asarray(inputs["x"], dtype=np.float32)
    B, H, W, Cc = x.shape
    xf = np.ascontiguousarray(x.reshape(B, H * W, Cc))
    qkv_w = np.ascontiguousarray(np.asarray(inputs["qkv_w"], dtype=np.float32))
    qkv_b = np.ascontiguousarray(np.asarray(inputs["qkv_b"], dtype=np.float32))
    proj_w = np.ascontiguousarray(np.asarray(inputs["proj_w"], dtype=np.float32))
    proj_b = np.ascontiguousarray(np.asarray(inputs["proj_b"], dtype=np.float32))

    nc = _get_nc()
    in_maps = [
        {
            "x": xf[b],
            "qkv_w": qkv_w,
            "qkv_b": qkv_b,
            "proj_w": proj_w,
            "proj_b": proj_b,
        }
        for b in range(B)
    ]
    res = run_bass_kernel_spmd(nc, in_maps, list(range(B)), trace=trace, tmpdir=tmpdir)
    out = np.stack([res.results[b]["out"] for b in range(B)])
    return out.reshape(B, H, W, Cc).astype(np.float32), res


def kernel(x, qkv_w, qkv_b, proj_w, proj_b):
    out, _ = run(
        {
            "x": x,
            "qkv_w": qkv_w,
            "qkv_b": qkv_b,
            "proj_w": proj_w,
            "proj_b": proj_b,
        }
    )
    return out



# revision 18
# speedup vs baseline: 1.2326x; 1.2326x over previous
"""Trainium2 Bass kernel for multi-head attention (nn_Attention).

Problem: x[8, 32, 32, 768] -> MHA(12 heads, d=64) -> out[8, 32, 32, 768].

Sharding: pure data parallel. Batch B=8 maps 1:1 onto the 8 NeuronCores;
weights are replicated. No collectives.

v2 design (vs the fp32-PE-transpose baseline at 397us):
  - All input transposes (x, qkv_w, proj_w) moved off the PE: DMA fp32 row
    tiles to SBUF, cast to bf16 on the idle GPSIMD engine, then SBUF->SBUF
    XBAR dma_start_transpose into [P, tile, ct, 128] layouts (each call's
    destination is per-partition contiguous, required by the XBAR path).
  - qkv bias loaded as one strided DMA into [128, 18] feature-major; the
    v-bias is folded into the proj bias (softmax rows sum to 1):
    pb' = pb + proj_w @ bv, computed with one tiny PE matmul chain.
  - Scores S^T = k^T.T @ q^T per head pair (2 heads packed in the 128-row
    PE array via tile_position), exp on ACT straight out of PSUM into bf16
    E tiles; PV accumulates [V|1].T @ E so the softmax denominator rides
    along as PSUM row 64.
  - Normalization never stalls the PE: reciprocal (DVE, direct from PSUM)
    is emitted right after the last PV matmul, an independent qkv
    projection unit runs on the PE while it completes, then the f32r
    ones-broadcast matmul + DVE multiply produce normalized OT.
  - Emission interleaves scores(p) / pv(p-1) per key tile with qkv/V
    projection filler units so the PE never sits on a single dependency
    and PSUM (2x 2-bank rotating pools + 2x 2-bank PV accumulators) always
    has a drained buffer ready.
"""

import os
import sys

for _p in ("/opt/trn_rl_repo",):
    if _p not in sys.path:
        sys.path.insert(0, _p)

import numpy as np

import concourse.bass as bass
from concourse import bacc
import concourse.mybir as mybir
from concourse.tile import TileContext

F32 = mybir.dt.float32
F32R = mybir.dt.float32r
BF16 = mybir.dt.bfloat16

P = 128
C = 768            # model dim
CT = C // P        # 6 c-tiles
N = 1024           # tokens per batch element
NT = N // P        # 8 token tiles
HEADS = 12
D = 64
PAIRS = HEADS // 2  # 6
OT3 = 3 * C // P   # 18 qkv_w row tiles
SCALE = D ** -0.5  # 0.125


def build_nc() -> bass.Bass:
    nc = bacc.Bacc(None, target_bir_lowering=False)
    x_d = nc.declare_dram_parameter("x", [N, C], F32, isOutput=False)
    qkvw_d = nc.declare_dram_parameter("qkv_w", [3 * C, C], F32, isOutput=False)
    qkvb_d = nc.declare_dram_parameter("qkv_b", [3 * C], F32, isOutput=False)
    projw_d = nc.declare_dram_parameter("proj_w", [C, C], F32, isOutput=False)
    projb_d = nc.declare_dram_parameter("proj_b", [C], F32, isOutput=False)
    out_d = nc.declare_dram_parameter("out", [N, C], F32, isOutput=True)

    with TileContext(nc) as tc:
        with (
            tc.tile_pool(name="const", bufs=1) as cpool,
            tc.tile_pool(name="load", bufs=3) as lpool,
            tc.tile_pool(name="cast", bufs=3) as bfpool,
            tc.tile_pool(name="qk", bufs=2) as qkpool,
            tc.tile_pool(name="v", bufs=1) as vpool,
            tc.tile_pool(name="otp", bufs=1) as otpool,
            tc.tile_pool(name="xTp", bufs=1) as xtpool,
            tc.tile_pool(name="wTp", bufs=1) as wtpool,
            tc.tile_pool(name="pwp", bufs=1) as pwpool,
            tc.tile_pool(name="e", bufs=4) as epool,
            tc.tile_pool(name="rec", bufs=2) as rpool,
            tc.tile_pool(name="bc", bufs=2) as bcpool,
            tc.tile_pool(name="outs", bufs=2) as outpool,
            tc.tile_pool(name="psa", bufs=2, space="PSUM") as psa,
            tc.tile_pool(name="psv", bufs=2, space="PSUM") as psv,
        ):
            # ---------------- persistent tensors ----------------
            xT = xtpool.tile([P, NT, CT, P], BF16, tag="xT")     # x^T per nt
            WT = wtpool.tile([P, OT3, CT, P], BF16, tag="WT")    # qkv_w^T per ot
            PwT = pwpool.tile([P, CT, CT, P], BF16, tag="PwT")   # proj_w^T per c2t
            V = vpool.tile([P, NT, HEADS, D + 1], BF16, tag="V")  # token-major + ones
            OT = otpool.tile([P, CT, N], BF16, tag="OT")         # attn out, feat-major
            QK = {}  # pair -> (q_tile, k_tile), feature-major [128, N]

            # ---------------- loads: x then first W tiles ----------------
            def load_cast_transpose(dram_rows, dest3):
                """[128, C] fp32 DMA -> gpsimd bf16 cast -> XBAR transpose to
                dest3 [128, CT, 128] (contiguous per partition)."""
                st = lpool.tile([P, C], F32, tag="ld")
                nc.sync.dma_start(st, dram_rows)
                bt = bfpool.tile([P, C], BF16, tag="cast")
                nc.gpsimd.tensor_copy(bt, st)
                nc.sync.dma_start_transpose(dest3, bt)

            def ldx(nt):
                load_cast_transpose(x_d[nt * P : (nt + 1) * P, :], xT[:, nt])

            def ldw(ot):
                load_cast_transpose(qkvw_d[ot * P : (ot + 1) * P, :], WT[:, ot])

            def ldpw(ct):
                load_cast_transpose(projw_d[ct * P : (ct + 1) * P, :], PwT[:, ct])

            for nt in range(NT):
                ldx(nt)
            ldw(0)
            ldw(6)
            ldw(12)

            # ---------------- constants & biases ----------------
            ones_st = cpool.tile([1, P], F32, tag="ones_st")
            nc.gpsimd.memset(ones_st, 1.0)
            ones_bf = cpool.tile([1, P], BF16, tag="ones_bf")
            nc.vector.tensor_copy(ones_bf, ones_st)
            ones_r = cpool.tile([1, P], F32R, tag="ones_r")
            nc.vector.tensor_copy(ones_r, ones_st)
            nc.gpsimd.memset(V[:, :, :, D], 1.0)

            # feature-major qkv bias [128, 18]; cols 0..11 = q,k; 12..17 = v
            biasT = cpool.tile([P, OT3], F32, tag="biasT")
            nc.sync.dma_start(biasT, qkvb_d.rearrange("(t p) -> p t", p=P))
            bvT = cpool.tile([P, CT], BF16, tag="bvT")
            nc.vector.tensor_copy(bvT, biasT[:, 2 * CT :])
            pb_st = cpool.tile([1, C], F32, tag="pb_st")
            nc.sync.dma_start(pb_st, projb_d[None, :])
            pbp = cpool.tile([1, C], BF16, tag="pbp")  # pb + Pw @ bv

            # ---------------- compute units ----------------
            def qk_proj(p, which):
                """Feature-major q (which=0) or k (which=1) projection for
                head pair p; allocates the pair's [128, N] tile."""
                ot = p + which * CT
                ps = psa.tile([P, N], F32, tag="psa", name=f"ps_qk{ot}")
                for ct in range(CT):
                    for ic in range(2):
                        nc.tensor.matmul(
                            ps[:, ic * 512 : (ic + 1) * 512],
                            WT[:, ot, ct, :],
                            xT[:, ic * 4 : ic * 4 + 4, ct, :],
                            start=(ct == 0),
                            stop=(ct == CT - 1),
                        )
                t = qkpool.tile(
                    [P, N], BF16, tag="kT" if which else "qT", name=f"qk{ot}"
                )
                QK.setdefault(p, [None, None])[which] = t
                nc.vector.tensor_scalar_add(t, ps, biasT[:, ot : ot + 1])

            def v_proj(vp, nt):
                """Token-major V for head pair vp, token tile nt (no bias --
                v bias is folded into the proj bias)."""
                ps = psa.tile([P, P], F32, tag="psa", name=f"ps_v{vp}_{nt}")
                for ct in range(CT):
                    nc.tensor.matmul(
                        ps,
                        xT[:, nt, ct, :],
                        WT[:, 2 * CT + vp, ct, :],
                        start=(ct == 0),
                        stop=(ct == CT - 1),
                    )
                nc.vector.tensor_copy(
                    V[:, nt, 2 * vp : 2 * vp + 2, 0:D],
                    ps.rearrange("p (h d) -> p h d", d=D),
                )

            def scores(pair, jt, half, E):
                lo = half * D
                qt, kt = QK[pair]
                ps = psa.tile([P, N], F32, tag="psa", name=f"ps_s{pair}_{jt}_{half}")
                for ic in range(2):
                    nc.tensor.matmul(
                        ps[:, ic * 512 : (ic + 1) * 512],
                        kt[lo : lo + D, jt * P : (jt + 1) * P],
                        qt[lo : lo + D, ic * 512 : (ic + 1) * 512],
                        start=True,
                        stop=True,
                        tile_position=(lo, 0),
                    )
                nc.scalar.activation(
                    E[:, jt, :], ps, mybir.ActivationFunctionType.Exp, scale=SCALE
                )

            def pv(h, jt, E, pspv):
                for ic in range(2):
                    nc.tensor.matmul(
                        pspv[0 : D + 1, ic * 512 : (ic + 1) * 512],
                        V[:, jt, h, :],
                        E[:, jt, ic * 512 : (ic + 1) * 512],
                        start=(jt == 0),
                        stop=(jt == NT - 1),
                    )

            def recip_den(h, pspv):
                den = rpool.tile([1, N], F32, tag="den", name=f"den{h}")
                nc.vector.tensor_copy(den, pspv[D : D + 1, :])
                rec_st = rpool.tile([1, N], F32, tag="rec_st", name=f"recs{h}")
                nc.vector.reciprocal_approx_fast(rec_st, den)
                rec = rpool.tile([1, N], F32R, tag="rec", name=f"rec{h}")
                nc.vector.tensor_copy(rec, rec_st)
                return rec

            def normalize(h, pspv, rec):
                """bcast 1/den across 64 partitions (f32r PE matmul), then
                OT[h] = pspv[0:D] * bcast on DVE."""
                psbc = psa.tile([D, N], F32, tag="psa", name=f"ps_bc{h}")
                for ic in range(2):
                    nc.tensor.matmul(
                        psbc[:, ic * 512 : (ic + 1) * 512],
                        ones_r[:, 0:D],
                        rec[:, ic * 512 : (ic + 1) * 512],
                        start=True,
                        stop=True,
                    )
                bcsb = bcpool.tile([D, N], BF16, tag="bc", name=f"bc{h}")
                nc.vector.tensor_copy(bcsb, psbc)
                nc.vector.tensor_mul(
                    OT[(h % 2) * D : (h % 2) * D + D, h // 2, :], pspv[0:D, :], bcsb
                )

            def pb_fold():
                """pbp = proj_b + proj_w @ v_bias (one [1, C] PE chain)."""
                ps = psa.tile([1, C], F32, tag="psa", name="ps_pb")
                for hdt in range(CT):
                    for o0, ow in ((0, 512), (512, 256)):
                        nc.tensor.matmul(
                            ps[:, o0 : o0 + ow],
                            bvT[:, hdt : hdt + 1],
                            PwT[:, o0 // P : (o0 + ow) // P, hdt, :],
                            start=(hdt == 0),
                            stop=(hdt == CT - 1),
                        )
                nc.vector.tensor_add(pbp, ps, pb_st)

            def proj(it):
                outt = outpool.tile([P, C], F32, tag="out")
                ps = psa.tile([P, N], F32, tag="psa", name=f"ps_o{it}")
                for o0, ow in ((0, 512), (512, 256)):
                    nc.tensor.matmul(
                        ps[:, o0 : o0 + ow], ones_bf, pbp[:, o0 : o0 + ow],
                        start=True, stop=False,
                    )
                for hdt in range(CT):
                    for o0, ow in ((0, 512), (512, 256)):
                        nc.tensor.matmul(
                            ps[:, o0 : o0 + ow],
                            OT[:, hdt, it * P : (it + 1) * P],
                            PwT[:, o0 // P : (o0 + ow) // P, hdt, :],
                            start=False,
                            stop=(hdt == CT - 1),
                        )
                nc.vector.tensor_copy(outt, ps[:, :C])
                nc.sync.dma_start(out_d[it * P : (it + 1) * P, :], outt)

            # ---------------- interleaved emission ----------------
            ldw(1)
            ldw(7)
            qk_proj(0, 0)
            qk_proj(0, 1)
            ldw(13)

            Es = {}

            def new_E(p):
                E0 = epool.tile([P, NT, N], BF16, tag="E", name=f"E0_{p}")
                E1 = epool.tile([P, NT, N], BF16, tag="E", name=f"E1_{p}")
                Es[p] = (E0, E1)

            # pair 0: scores only, with qk(1)/qk(7)/v_proj(0) as filler
            new_E(0)

            def ld28():
                ldw(2)
                ldw(8)

            fill0 = (
                [lambda: qk_proj(1, 0), lambda: qk_proj(1, 1)]
                + [lambda nt=nt: v_proj(0, nt) for nt in range(4)]
                + [ld28]
            )
            for jt in range(NT):
                scores(0, jt, 0, Es[0][0])
                scores(0, jt, 1, Es[0][1])
                if jt < len(fill0):
                    fill0[jt]()
            for nt in range(4, NT):
                v_proj(0, nt)

            # pairs 1..5: scores(p) loop, then pv(p-1) as contiguous
            # accumulation blocks (HW PSUM groups must not interleave)
            W_SCHED = {1: (3, 9, 14), 2: (4, 10, 15), 3: (5, 11, 16), 4: (17,)}
            for p in range(1, PAIRS):
                new_E(p)
                hA, hB = 2 * (p - 1), 2 * (p - 1) + 1
                for jt in range(NT):
                    scores(p, jt, 0, Es[p][0])
                    scores(p, jt, 1, Es[p][1])
                    if jt < 7:
                        v_proj(p, jt)
                    if jt == 0 and p in W_SCHED:
                        for ot in W_SCHED[p]:
                            ldw(ot)
                    if jt == 1 and p == 1:
                        for ct in range(CT):
                            ldpw(ct)
                pspvA = psv.tile([D + 1, N], F32, tag="psv", name=f"pvA{p}")
                for jt in range(NT):
                    pv(hA, jt, Es[p - 1][0], pspvA)
                recA = recip_den(hA, pspvA)
                pspvB = psv.tile([D + 1, N], F32, tag="psv", name=f"pvB{p}")
                for jt in range(NT):
                    pv(hB, jt, Es[p - 1][1], pspvB)
                recB = recip_den(hB, pspvB)
                # independent PE work while the reciprocals complete
                if p < PAIRS - 1:
                    qk_proj(p + 1, 0)
                else:
                    pb_fold()
                normalize(hA, pspvA, recA)
                if p < PAIRS - 1:
                    qk_proj(p + 1, 1)
                normalize(hB, pspvB, recB)
                v_proj(p, 7)

            # final pair's PV + normalize, then output projection
            pspvA = psv.tile([D + 1, N], F32, tag="psv", name="pvA6")
            for jt in range(NT):
                pv(10, jt, Es[5][0], pspvA)
            recA = recip_den(10, pspvA)
            pspvB = psv.tile([D + 1, N], F32, tag="psv", name="pvB6")
            for jt in range(NT):
                pv(11, jt, Es[5][1], pspvB)
            recB = recip_den(11, pspvB)
            normalize(10, pspvA, recA)
            normalize(11, pspvB, recB)
            for it in range(NT):
                proj(it)

    nc.compile()
    return nc


_NC_CACHE = None


def _get_nc():
    global _NC_CACHE
    if _NC_CACHE is None:
        _NC_CACHE = build_nc()
    return _NC_CACHE


def run(inputs, trace=False, tmpdir=None):
    """Run on 8 NeuronCores; returns (out[8,32,32,768], BassKernelResults)."""
    from concourse.bass_utils import run_bass_kernel_spmd

    x = np.asarray(inputs["x"], dtype=np.float32)
    B, H, W, Cc = x.shape
    xf = np.ascontiguousarray(x.reshape(B, H * W, Cc))
    qkv_w = np.ascontiguousarray(np.asarray(inputs["qkv_w"], dtype=np.float32))
    qkv_b = np.ascontiguousarray(np.asarray(inputs["qkv_b"], dtype=np.float32))
    proj_w = np.ascontiguousarray(np.asarray(inputs["proj_w"], dtype=np.float32))
    proj_b = np.ascontiguousarray(np.asarray(inputs["proj_b"], dtype=np.float32))

    nc = _get_nc()
    in_maps = [
        {
            "x": xf[b],
            "qkv_w": qkv_w,
            "qkv_b": qkv_b,
            "proj_w": proj_w,
            "proj_b": proj_b,
        }
        for b in range(B)
    ]
    res = run_bass_kernel_spmd(nc, in_maps, list(range(B)), trace=trace, tmpdir=tmpdir)
    out = np.stack([res.results[b]["out"] for b in range(B)])
    return out.reshape(B, H, W, Cc).astype(np.float32), res


def kernel(x, qkv_w, qkv_b, proj_w, proj_b):
    out, _ = run(
        {
            "x": x,
            "qkv_w": qkv_w,
            "qkv_b": qkv_b,
            "proj_w": proj_w,
            "proj_b": proj_b,
        }
    )
    return out
